# revision 1
# baseline (speedup 1.0000x reference)
"""Bass/Trainium2 kernel for nn_BakaAttention: 8-way data-parallel over batch.

Per core (one batch element):
  q = rope(x@wq, off=1024); k = rope(concat(past_k, x@wk), off=0); v = concat(past_v, x@wv)
  out = softmax(mask(q k^T / 16)) v @ wo

Layouts on chip: qT/kT are feature-major [f, t]; scores computed transposed
[s, t] so PV consumes probs directly as the stationary operand; softmax
row-sums ride along as a 257th "ones" column of the moving v operand.
All matmuls run in float32r (full PE rate at N>=256, ~1e-4 rel err).
"""

import numpy as np

B, T, P, H, DH, DIN, DOUT = 8, 1024, 1024, 4, 256, 1024, 1152
S = P + T  # 2048 keys
THETA = 10000.0
NCORES = 8


def _host_constants():
    m = np.arange(0, DH, 2, dtype=np.float64) / DH          # 128 freqs
    inv = 1.0 / (THETA ** m)                                # [128]
    pos = np.arange(S, dtype=np.float64)                    # [2048]
    ang = np.outer(inv, pos)                                # [128, 2048]
    cos_full = np.cos(ang)
    sin_full = np.sin(ang)
    r = np.arange(128) // 2
    consts = {
        "cos_lo": cos_full[r, :].astype(np.float32),
        "cos_hi": cos_full[64 + r, :].astype(np.float32),
        "sin_lo": sin_full[r, :].astype(np.float32),
        "sin_hi": sin_full[64 + r, :].astype(np.float32),
    }
    prot = np.zeros((128, 128), np.float32)
    for mm in range(64):
        prot[2 * mm, 2 * mm + 1] = 1.0
        prot[2 * mm + 1, 2 * mm] = -1.0
    consts["prot"] = prot
    consts["ident"] = np.eye(128, dtype=np.float32)
    # masks[ci][sl, tl] = 1.0 if sl <= tl - 128*ci else 0 (keep), ci in 0..3
    sl = np.arange(128)[:, None]
    tl = np.arange(512)[None, :]
    masks = np.stack(
        [(sl <= tl - 128 * ci).astype(np.float32) for ci in range(4)], axis=1
    )  # [128, 4, 512]
    consts["masks"] = np.ascontiguousarray(masks)
    consts["ones"] = np.ones((128, 4), np.float32)
    consts["onesr"] = np.ones((1, 128), np.float32)
    return consts


def build_kernel(debug=False):
    import concourse.bass as bass
    import concourse.mybir as mybir
    from concourse import bacc
    from concourse.tile import TileContext

    f32 = mybir.dt.float32
    f32r = mybir.dt.float32r
    AF = mybir.ActivationFunctionType
    OP = mybir.AluOpType

    nc = bacc.Bacc(None, target_bir_lowering=False)

    x_d = nc.dram_tensor("x", [T, DIN], f32r, kind="ExternalInput")
    pk_d = nc.dram_tensor("past_k", [P, H, DH], f32r, kind="ExternalInput")
    pv_d = nc.dram_tensor("past_v", [P, H, DH], f32r, kind="ExternalInput")
    wq_d = nc.dram_tensor("wq", [DIN, DIN], f32r, kind="ExternalInput")
    wk_d = nc.dram_tensor("wk", [DIN, DIN], f32r, kind="ExternalInput")
    wv_d = nc.dram_tensor("wv", [DIN, DIN], f32r, kind="ExternalInput")
    wo_d = nc.dram_tensor("wo", [DIN, DOUT], f32r, kind="ExternalInput")
    cos_lo_d = nc.dram_tensor("cos_lo", [128, S], f32, kind="ExternalInput")
    cos_hi_d = nc.dram_tensor("cos_hi", [128, S], f32, kind="ExternalInput")
    sin_lo_d = nc.dram_tensor("sin_lo", [128, S], f32, kind="ExternalInput")
    sin_hi_d = nc.dram_tensor("sin_hi", [128, S], f32, kind="ExternalInput")
    prot_d = nc.dram_tensor("prot", [128, 128], f32r, kind="ExternalInput")
    ident_d = nc.dram_tensor("ident", [128, 128], f32r, kind="ExternalInput")
    masks_d = nc.dram_tensor("masks", [128, 4, 512], f32, kind="ExternalInput")
    ones_d = nc.dram_tensor("ones", [128, 4], f32r, kind="ExternalInput")
    onesr_d = nc.dram_tensor("onesr", [1, 128], f32r, kind="ExternalInput")
    out_d = nc.dram_tensor("out", [T, DOUT], f32, kind="ExternalOutput")
    vkind = dict(kind="ExternalOutput") if debug else {}
    v_r = nc.dram_tensor("v_r", [T, DIN], f32r, **vkind)
    qT_r = nc.dram_tensor("qT_r", [8, 128, T], f32r, **vkind)
    if debug:
        kT_dump = nc.dram_tensor("kT_dump", [8, 128, S], f32r, kind="ExternalOutput")
        y_dump = nc.dram_tensor("y_dump", [8, 128, DIN], f32, kind="ExternalOutput")

    from contextlib import ExitStack
    stack = ExitStack()
    with TileContext(nc) as tc, stack:
        cstp = stack.enter_context(tc.tile_pool(name="consts", bufs=1))
        prot = cstp.tile([128, 128], f32r, name="prot", tag="prot")
        ident = cstp.tile([128, 128], f32r, name="ident", tag="ident")
        masks = cstp.tile([128, 4, 512], f32, name="masks", tag="masks")
        ones_sb = cstp.tile([128, 4], f32r, name="ones_sb", tag="ones_sb")
        nc.sync.dma_start(out=ones_sb[:], in_=ones_d[:])
        onesr_sb = cstp.tile([1, 128], f32r, name="onesr_sb", tag="onesr_sb")
        nc.sync.dma_start(out=onesr_sb[:], in_=onesr_d[:])
        nc.sync.dma_start(out=prot[:], in_=prot_d[:])
        nc.sync.dma_start(out=ident[:], in_=ident_d[:])
        nc.sync.dma_start(out=masks[:], in_=masks_d[:])

        resid = stack.enter_context(tc.tile_pool(name="resid", bufs=1))
        kT = [resid.tile([128, S], f32r, name=f"kT{i}", tag=f"kT{i}") for i in range(8)]

        # ---------------- Phase 1+2: xT, projections, rope ----------------
        with tc.tile_pool(name="tables", bufs=1) as tabp, \
             tc.tile_pool(name="p2xT", bufs=1) as xtp, \
             tc.tile_pool(name="p2", bufs=2) as p2p, \
             tc.tile_pool(name="p2st", bufs=3) as stp, \
             tc.tile_pool(name="p2ps", bufs=4, space="PSUM") as ps2, \
             tc.tile_pool(name="p2rot", bufs=2, space="PSUM") as rotps, \
             tc.tile_pool(name="p2kp", bufs=1) as kpp:
            cos_t = [tabp.tile([128, T], f32, name="clo", tag="clo"),
                     tabp.tile([128, T], f32, name="chi", tag="chi")]
            sin_t = [tabp.tile([128, T], f32, name="slo", tag="slo"),
                     tabp.tile([128, T], f32, name="shi", tag="shi")]

            def load_tables(p0):
                nc.sync.dma_start(out=cos_t[0][:], in_=cos_lo_d[:, p0:p0 + T])
                nc.sync.dma_start(out=cos_t[1][:], in_=cos_hi_d[:, p0:p0 + T])
                nc.sync.dma_start(out=sin_t[0][:], in_=sin_lo_d[:, p0:p0 + T])
                nc.sync.dma_start(out=sin_t[1][:], in_=sin_hi_d[:, p0:p0 + T])

            load_tables(P)  # positions 1024..2047 for q and new-k

            xT = [xtp.tile([128, T], f32r, name=f"xT{i}", tag=f"xT{i}") for i in range(8)]
            for tt in range(8):
                xt = p2p.tile([128, DIN], f32r, name="xload", tag="xload")
                nc.sync.dma_start(out=xt[:], in_=x_d[128 * tt:128 * (tt + 1), :])
                for kt in range(8):
                    tp = ps2.tile([128, 128], f32, name="tps", tag="tps", bufs=2)
                    nc.tensor.matmul(tp[:], xt[:, 128 * kt:128 * (kt + 1)], ident[:],
                                     start=True, stop=True)
                    nc.scalar.copy(xT[kt][:, 128 * tt:128 * (tt + 1)], tp[:])

            def rope_combine(dst_ap, raw_sb, rot_ps, ft, off, n):
                # dst = raw * cos + rot * sin ; table rows by f-tile parity
                ctab = cos_t[ft % 2][:, off:off + n]
                stab = sin_t[ft % 2][:, off:off + n]
                t1 = p2p.tile([128, 512], f32, name="ropet1", tag="ropet1")
                nc.gpsimd.tensor_tensor(t1[:, :n], raw_sb, ctab, op=OP.mult)
                t2 = p2p.tile([128, 512], f32, name="ropet2", tag="ropet2")
                nc.vector.tensor_tensor(t2[:, :n], rot_ps, stab, op=OP.mult)
                nc.vector.tensor_tensor(dst_ap, t1[:, :n], t2[:, :n], op=OP.add)

            # q and new-k projections (transposed layout) + rope
            for w_d, dst in ((wq_d, None), (wk_d, kT)):
                for ftg in range(4):          # pairs of f-tiles
                    psl = [ps2.tile([128, 512], f32, name=f"pj{i}", tag=f"pj{i}", bufs=1) for i in range(4)]
                    for kt in range(8):
                        wt = stp.tile([128, 256], f32r, name="wload", tag="wload")
                        nc.sync.dma_start(
                            out=wt[:],
                            in_=w_d[128 * kt:128 * (kt + 1), 256 * ftg:256 * (ftg + 1)])
                        for f2 in range(2):
                            for th in range(2):
                                nc.tensor.matmul(
                                    psl[2 * f2 + th][:],
                                    wt[:, 128 * f2:128 * (f2 + 1)].bitcast(f32r),
                                    xT[kt][:, 512 * th:512 * (th + 1)].bitcast(f32r),
                                    start=(kt == 0), stop=(kt == 7))
                    for f2 in range(2):
                        ft = 2 * ftg + f2
                        raw = p2p.tile([128, 1024], f32r, name="rawsb", tag="rawsb")
                        for th in range(2):
                            nc.scalar.copy(raw[:, 512 * th:512 * (th + 1)],
                                           psl[2 * f2 + th][:])
                        if dst is None:
                            qstage = p2p.tile([128, 1024], f32r, name="qstage",
                                              tag="qstage")
                        for th in range(2):
                            rp = rotps.tile([128, 512], f32, name="rotps", tag="rotps")
                            nc.tensor.matmul(rp[:], prot[:].bitcast(f32r),
                                             raw[:, 512 * th:512 * (th + 1)].bitcast(f32r),
                                             start=True, stop=True)
                            if dst is None:
                                dst_ap = qstage[:, 512 * th:512 * (th + 1)]
                            else:
                                dst_ap = dst[ft][:, P + 512 * th:P + 512 * (th + 1)]
                            rope_combine(dst_ap, raw[:, 512 * th:512 * (th + 1)],
                                         rp[:], ft, 512 * th, 512)
                        if dst is None:
                            nc.sync.dma_start(out=qT_r[ftg * 2 + f2], in_=qstage[:])

            # v projection, natural layout [s, f] -> DRAM
            for stg in range(4):
                psl = [ps2.tile([128, 512], f32, name=f"pv{i}", tag=f"pj{i}", bufs=1) for i in range(4)]
                for kt in range(8):
                    wt = stp.tile([128, 1024], f32r, name="wvload", tag="wvload")
                    nc.sync.dma_start(out=wt[:], in_=wv_d[128 * kt:128 * (kt + 1), :])
                    for s2 in range(2):
                        st = 2 * stg + s2
                        for fh in range(2):
                            nc.tensor.matmul(
                                psl[2 * s2 + fh][:],
                                xT[kt][:, 128 * st:128 * (st + 1)].bitcast(f32r),
                                wt[:, 512 * fh:512 * (fh + 1)].bitcast(f32r),
                                start=(kt == 0), stop=(kt == 7))
                for s2 in range(2):
                    st = 2 * stg + s2
                    vsb = p2p.tile([128, 1024], f32r, name="vsb", tag="vsb")
                    for fh in range(2):
                        nc.scalar.copy(vsb[:, 512 * fh:512 * (fh + 1)],
                                       psl[2 * s2 + fh][:])
                    nc.sync.dma_start(out=v_r[128 * st:128 * (st + 1), :], in_=vsb[:])

            # past_k: transpose + rope into kT[:, 0:1024]
            load_tables(0)  # positions 0..1023
            for h in range(4):
                kp = [kpp.tile([128, P], f32r, name=f"kp{i}", tag=f"kp{i}") for i in range(2)]
                for st in range(8):
                    pkt = stp.tile([128, DH], f32r, name="pkload", tag="pkload")
                    nc.sync.dma_start(out=pkt[:],
                                      in_=pk_d[128 * st:128 * (st + 1), h, :])
                    for f2 in range(2):
                        tp = ps2.tile([128, 128], f32, name="tps", tag="tps", bufs=2)
                        nc.tensor.matmul(tp[:], pkt[:, 128 * f2:128 * (f2 + 1)],
                                         ident[:], start=True, stop=True)
                        nc.scalar.copy(kp[f2][:, 128 * st:128 * (st + 1)], tp[:])
                for f2 in range(2):
                    ft = 2 * h + f2
                    for sh in range(2):
                        rp = rotps.tile([128, 512], f32, name="rotps", tag="rotps")
                        nc.tensor.matmul(rp[:], prot[:].bitcast(f32r),
                                         kp[f2][:, 512 * sh:512 * (sh + 1)].bitcast(f32r),
                                         start=True, stop=True)
                        rope_combine(kT[ft][:, 512 * sh:512 * (sh + 1)],
                                     kp[f2][:, 512 * sh:512 * (sh + 1)],
                                     rp[:], ft, 512 * sh, 512)

        if debug:
            for i in range(8):
                nc.sync.dma_start(out=kT_dump[i], in_=kT[i][:])

        # ---------------- Phase 3: attention ----------------
        ysbp = stack.enter_context(tc.tile_pool(name="ysb", bufs=1))
        yT = [ysbp.tile([128, T], f32r, name=f"yT{i}", tag=f"yT{i}")
              for i in range(8)]
        with tc.tile_pool(name="vaug", bufs=1) as vap, \
             tc.tile_pool(name="qth", bufs=2) as qtp, \
             tc.tile_pool(name="probs", bufs=5) as prp, \
             tc.tile_pool(name="p3sm", bufs=4) as smp, \
             tc.tile_pool(name="p3sc", bufs=3, space="PSUM") as scps, \
             tc.tile_pool(name="p3y", bufs=1, space="PSUM") as yps:
            for h in range(4):
                qh = [qtp.tile([128, T], f32r, name=f"qh{fk}", tag=f"qh{fk}")
                      for fk in range(2)]
                for fk in range(2):
                    nc.sync.dma_start(out=qh[fk][:], in_=qT_r[2 * h + fk])
                va = [vap.tile([128, 260], f32r, name=f"va{j}", tag=f"va{j}")
                      for j in range(16)]
                for j in range(16):
                    if j < 8:
                        src = pv_d[128 * j:128 * (j + 1), h, :]
                    else:
                        src = v_r[128 * (j - 8):128 * (j - 7),
                                  DH * h:DH * (h + 1)]
                    nc.sync.dma_start(out=va[j][:, 0:DH], in_=src)
                for TH in range(2):
                    jmax = 12 + 4 * TH
                    ytp_ps = [yps.tile([128, 512], f32, name=f"ytp{i}",
                                       tag=f"ytp{i}", bufs=1) for i in range(2)]
                    sm_ps = yps.tile([1, 512], f32, name="smps", tag="smps",
                                     bufs=1)
                    for j in range(jmax):
                        sc = scps.tile([128, 512], f32, name="sc", tag="sc")
                        for fk in range(2):
                            nc.tensor.matmul(
                                sc[:],
                                kT[2 * h + fk][:, 128 * j:128 * (j + 1)].bitcast(f32r),
                                qh[fk][:, 512 * TH:512 * (TH + 1)].bitcast(f32r),
                                start=(fk == 0), stop=(fk == 1))
                        pj = prp.tile([128, 512], f32r, name="pj", tag="pj")
                        nc.scalar.activation(pj[:], sc[:], AF.Exp, scale=float(DH ** -0.5))
                        ci = j - (8 + 4 * TH)
                        if ci >= 0:
                            nc.gpsimd.tensor_tensor(pj[:], pj[:], masks[:, ci, :],
                                                    op=OP.mult)
                        for fb in range(2):
                            nc.tensor.matmul(
                                ytp_ps[fb][:],
                                va[j][:, 128 * fb:128 * (fb + 1)],
                                pj[:],
                                start=(j == 0), stop=(j == jmax - 1))
                        nc.tensor.matmul(
                            sm_ps[:], ones_sb[:, 0:1], pj[:],
                            start=(j == 0), stop=(j == jmax - 1))
                    rc = smp.tile([1, 512], f32r, name="rc", tag="rc")
                    with nc.allow_low_precision(reason="f32r bits == f32"):
                        nc.vector.reciprocal(rc[:], sm_ps[:])
                    bc_ps = scps.tile([128, 512], f32, name="bcps", tag="bcps",
                                      bufs=1)
                    nc.tensor.matmul(bc_ps[:], onesr_sb[:], rc[:],
                                     start=True, stop=True)
                    bc_sb = smp.tile([128, 512], f32, name="bcsb", tag="bcsb")
                    nc.scalar.copy(bc_sb[:], bc_ps[:])
                    for fb in range(2):
                        nc.vector.tensor_tensor(
                            yT[2 * h + fb][:, 512 * TH:512 * (TH + 1)],
                            ytp_ps[fb][:],
                            bc_sb[:],
                            op=OP.mult)

        # ---------------- Phase 4: o-projection ----------------
        with tc.tile_pool(name="p4wo", bufs=1) as wop, \
             tc.tile_pool(name="p4o", bufs=2) as osp, \
             tc.tile_pool(name="p4ps", bufs=4, space="PSUM") as ps4:
            wo_sb = [wop.tile([128, DOUT], f32r, name=f"wo{i}", tag=f"wo{i}")
                     for i in range(8)]
            for kt in range(8):
                nc.sync.dma_start(out=wo_sb[kt][:],
                                  in_=wo_d[128 * kt:128 * (kt + 1), :])
            for tt in range(8):
                ot = osp.tile([128, DOUT], f32, name="osb", tag="osb")
                for ds in range(3):
                    op_ps = ps4.tile([128, 384], f32, name="ops", tag="ops", bufs=3)
                    for fk in range(8):
                        nc.tensor.matmul(
                            op_ps[:],
                            yT[fk][:, 128 * tt:128 * (tt + 1)],
                            wo_sb[fk][:, 384 * ds:384 * (ds + 1)],
                            start=(fk == 0), stop=(fk == 7))
                    nc.scalar.copy(ot[:, 384 * ds:384 * (ds + 1)], op_ps[:])
                nc.sync.dma_start(out=out_d[128 * tt:128 * (tt + 1), :], in_=ot[:])

    nc.finalize()
    return nc


_NC_CACHE = {}


def run(x, past_k, past_v, wq, wk, wv, wo, debug=False, trace=False):
    from concourse.bass_utils import run_bass_kernel_spmd

    key = (debug,)
    if key not in _NC_CACHE:
        _NC_CACHE[key] = build_kernel(debug=debug)
    nc = _NC_CACHE[key]
    consts = _host_constants()
    in_maps = []
    for b in range(NCORES):
        m = {
            "x": np.ascontiguousarray(x[b]),
            "past_k": np.ascontiguousarray(past_k[b]),
            "past_v": np.ascontiguousarray(past_v[b]),
            "wq": wq, "wk": wk, "wv": wv, "wo": wo,
            "cos_lo": consts["cos_lo"], "cos_hi": consts["cos_hi"],
            "sin_lo": consts["sin_lo"], "sin_hi": consts["sin_hi"],
            "prot": consts["prot"], "ident": consts["ident"],
            "masks": consts["masks"], "ones": consts["ones"], "onesr": consts["onesr"],
        }
        in_maps.append(m)
    res = run_bass_kernel_spmd(nc, in_maps, list(range(NCORES)), trace=trace)
    out = np.stack([res.results[b]["out"] for b in range(NCORES)], axis=0)
    return out, res


def kernel(x, past_k, past_v, wq, wk, wv, wo):
    out, _ = run(x, past_k, past_v, wq, wk, wv, wo)
    return out



# revision 12
# speedup vs baseline: 1.4585x; 1.4585x over previous
"""Bass/Trainium2 kernel for nn_BakaAttention: 8-way data-parallel over batch.

Per core (one batch element):
  q = rope(x@wq, off=1024); k = rope(concat(past_k, x@wk), off=0); v = x@wv
  out = softmax(mask(q k^T / 16)) [past_v; v] @ wo

Host-side prep (outside HW time): x pre-transposed to [din, t]; wq/wk
columns and past_k features permuted so rope interleaved pairs (2m,2m+1)
land at row m of adjacent feature tiles -> rope is pure elementwise DVE
work with one shared cos/sin table, no PE rotation. All matmul operands
cast to bf16 (streams at 1 col/cycle like f32r, halves SBUF/DMA).

On chip: everything SBUF-resident. Scores computed transposed [keys, q]
so probs feed PV directly as the moving operand; softmax denominators
accumulate on the Vector engine (pacc += pj) with a single [128,1]-ones
matmul per group instead of a per-chunk PE row-sum. Causal structure is
exploited at 128-query granularity: key chunk j only streams the queries
that attend to it, and only the diagonal 128x128 block gets masked.
"""

import numpy as np

B, T, P, H, DH, DIN, DOUT = 8, 1024, 1024, 4, 256, 1024, 1152
S = P + T  # 2048 keys
THETA = 10000.0
NCORES = 8


def _host_constants():
    m = np.arange(128, dtype=np.float64)
    inv = 1.0 / (THETA ** (2.0 * m / DH))                   # [128]
    pos = np.arange(S, dtype=np.float64)                    # [2048]
    ang = np.outer(inv, pos)                                # [128, 2048]
    tri = (np.arange(128)[:, None] <= np.arange(128)[None, :]).astype(np.float32)
    return {
        "cos": np.cos(ang).astype(np.float32),
        "sin": np.sin(ang).astype(np.float32),
        "tri": tri,  # cast to bf16 at pack time
        "ones": np.ones((128, 1), np.float32),
        "onesr": np.ones((1, 128), np.float32),
    }


def _perm():
    # per-head feature permutation: [evens, odds]
    p = np.empty(DIN, np.int64)
    for h in range(H):
        base = DH * h
        p[base:base + 128] = base + 2 * np.arange(128)
        p[base + 128:base + 256] = base + 2 * np.arange(128) + 1
    return p


def build_kernel():
    import concourse.bass as bass
    import concourse.mybir as mybir
    from concourse import bacc
    from concourse.tile import TileContext

    f32 = mybir.dt.float32
    f32r = mybir.dt.float32r
    bf16 = mybir.dt.bfloat16
    AF = mybir.ActivationFunctionType
    OP = mybir.AluOpType

    nc = bacc.Bacc(None, target_bir_lowering=False)

    xT_d = nc.dram_tensor("xT", [DIN, T], bf16, kind="ExternalInput")
    pkT_d = nc.dram_tensor("pkT", [8, 128, P], bf16, kind="ExternalInput")
    pv_d = nc.dram_tensor("pv", [P, DIN], bf16, kind="ExternalInput")
    wq_d = nc.dram_tensor("wq", [DIN, DIN], bf16, kind="ExternalInput")
    wk_d = nc.dram_tensor("wk", [DIN, DIN], bf16, kind="ExternalInput")
    wv_d = nc.dram_tensor("wv", [DIN, DIN], bf16, kind="ExternalInput")
    wo_d = nc.dram_tensor("wo", [DIN, DOUT], bf16, kind="ExternalInput")
    cos_d = nc.dram_tensor("cos", [128, S], f32, kind="ExternalInput")
    sin_d = nc.dram_tensor("sin", [128, S], f32, kind="ExternalInput")
    tri_d = nc.dram_tensor("tri", [128, 128], bf16, kind="ExternalInput")
    ones_d = nc.dram_tensor("ones", [128, 1], f32r, kind="ExternalInput")
    onesr_d = nc.dram_tensor("onesr", [1, 128], f32, kind="ExternalInput")
    out_d = nc.dram_tensor("out", [T, DOUT], f32, kind="ExternalOutput")

    from contextlib import ExitStack
    stack = ExitStack()
    with TileContext(nc) as tc, stack:
        # ---------------- persistent SBUF ----------------
        cstp = stack.enter_context(tc.tile_pool(name="consts", bufs=1))
        cos_t = cstp.tile([128, S], f32, name="cos", tag="cos")
        sin_t = cstp.tile([128, S], f32, name="sin", tag="sin")
        tri = cstp.tile([128, 128], bf16, name="tri", tag="tri")
        ones_sb = cstp.tile([128, 1], f32r, name="ones", tag="ones")
        onesr_sb = cstp.tile([1, 128], f32, name="onesr", tag="onesr")
        nc.sync.dma_start(out=cos_t[:], in_=cos_d[:])
        nc.sync.dma_start(out=sin_t[:], in_=sin_d[:])
        nc.sync.dma_start(out=tri[:], in_=tri_d[:])
        nc.sync.dma_start(out=ones_sb[:], in_=ones_d[:])
        nc.sync.dma_start(out=onesr_sb[:], in_=onesr_d[:])

        resid = stack.enter_context(tc.tile_pool(name="resid", bufs=1))
        xT = [resid.tile([128, T], bf16, name=f"xT{i}", tag=f"xT{i}")
              for i in range(8)]
        kT = [resid.tile([128, S], bf16, name=f"kT{i}", tag=f"kT{i}")
              for i in range(8)]
        qh = [resid.tile([128, T], bf16, name=f"qh{i}", tag=f"qh{i}")
              for i in range(8)]
        v_sb = [resid.tile([128, DIN], bf16, name=f"v{i}", tag=f"v{i}")
                for i in range(8)]
        pv_sb = [resid.tile([128, DIN], bf16, name=f"pv{i}", tag=f"pv{i}")
                 for i in range(8)]
        pkraw = [resid.tile([128, P], bf16, name=f"pkr{i}", tag=f"pkr{i}")
                 for i in range(8)]
        yT = [resid.tile([128, T], bf16, name=f"yT{i}", tag=f"yT{i}")
              for i in range(8)]
        wo_sb = [resid.tile([128, DOUT], bf16, name=f"wo{i}", tag=f"wo{i}")
                 for i in range(8)]

        for i in range(8):
            nc.sync.dma_start(out=xT[i][:], in_=xT_d[128 * i:128 * (i + 1), :])
        for i in range(8):
            nc.sync.dma_start(out=pkraw[i][:], in_=pkT_d[i])

        # past-k rope on gpsimd (independent of PE; runs under projections)
        # pair (A=tile 2h, B=tile 2h+1): kA = A*cos - B*sin; kB = B*cos + A*sin
        def past_rope(h):
            A, Bt = pkraw[2 * h], pkraw[2 * h + 1]
            c = cos_t[:, 0:P]
            s = sin_t[:, 0:P]
            t1 = ropep.tile([128, P], f32, name="prt1", tag="prt1")
            t2 = ropep.tile([128, P], f32, name="prt2", tag="prt2")
            nc.gpsimd.tensor_tensor(t1[:], A[:], c, op=OP.mult)
            nc.gpsimd.tensor_tensor(t2[:], Bt[:], s, op=OP.mult)
            nc.gpsimd.tensor_tensor(kT[2 * h][:, 0:P], t1[:], t2[:],
                                    op=OP.subtract)
            nc.gpsimd.tensor_tensor(t1[:], Bt[:], c, op=OP.mult)
            nc.gpsimd.tensor_tensor(t2[:], A[:], s, op=OP.mult)
            nc.gpsimd.tensor_tensor(kT[2 * h + 1][:, 0:P], t1[:], t2[:],
                                    op=OP.add)

        # ---------------- Phase 1: q/k proj + rope ----------------
        with tc.tile_pool(name="p1w", bufs=3) as wtp, \
             tc.tile_pool(name="p1rope", bufs=2) as ropep, \
             tc.tile_pool(name="p1ps", bufs=2, space="PSUM") as ps1:
            for wi, (w_d, dst, doff) in enumerate(
                    ((wq_d, qh, 0), (wk_d, kT, P))):
                for ftg in range(4):  # pair of f-tiles (one head)
                    psl = [ps1.tile([128, 512], f32, name=f"pj{i}",
                                    tag=f"pj{i}") for i in range(4)]
                    for kt in range(8):
                        wt = wtp.tile([128, 256], bf16, name="wld", tag="wld")
                        nc.sync.dma_start(
                            out=wt[:],
                            in_=w_d[128 * kt:128 * (kt + 1),
                                    256 * ftg:256 * (ftg + 1)])
                        for f2 in range(2):
                            for th in range(2):
                                nc.tensor.matmul(
                                    psl[2 * f2 + th][:],
                                    wt[:, 128 * f2:128 * (f2 + 1)],
                                    xT[kt][:, 512 * th:512 * (th + 1)],
                                    start=(kt == 0), stop=(kt == 7))
                    # rope combine on vector: raw pair (lo=psl[0/1], hi=psl[2/3])
                    c = cos_t[:, P:P + T]
                    s = sin_t[:, P:P + T]
                    for th in range(2):
                        sl = slice(512 * th, 512 * (th + 1))
                        A, Bt = psl[th][:], psl[2 + th][:]
                        t1 = ropep.tile([128, 512], f32, name="rt1", tag="rt1")
                        t2 = ropep.tile([128, 512], f32, name="rt2", tag="rt2")
                        nc.vector.tensor_tensor(t1[:], A, c[:, sl], op=OP.mult)
                        nc.vector.tensor_tensor(t2[:], Bt, s[:, sl], op=OP.mult)
                        nc.vector.tensor_tensor(
                            dst[2 * ftg][:, doff + 512 * th:doff + 512 * (th + 1)],
                            t1[:], t2[:], op=OP.subtract)
                        t3 = ropep.tile([128, 512], f32, name="rt3", tag="rt3")
                        t4 = ropep.tile([128, 512], f32, name="rt4", tag="rt4")
                        nc.vector.tensor_tensor(t3[:], Bt, c[:, sl], op=OP.mult)
                        nc.vector.tensor_tensor(t4[:], A, s[:, sl], op=OP.mult)
                        nc.vector.tensor_tensor(
                            dst[2 * ftg + 1][:, doff + 512 * th:doff + 512 * (th + 1)],
                            t3[:], t4[:], op=OP.add)
                    # interleave past-k rope (gpsimd) among q-proj groups
                    if wi == 0:
                        past_rope(ftg)

            # ---------------- Phase 2: v proj ----------------
            for stg in range(4):
                psl = [ps1.tile([128, 512], f32, name=f"pv{i}", tag=f"pj{i}")
                       for i in range(4)]
                for kt in range(8):
                    wt = wtp.tile([128, 1024], bf16, name="wvld", tag="wvld")
                    nc.sync.dma_start(out=wt[:],
                                      in_=wv_d[128 * kt:128 * (kt + 1), :])
                    for s2 in range(2):
                        st = 2 * stg + s2
                        for fh in range(2):
                            nc.tensor.matmul(
                                psl[2 * s2 + fh][:],
                                xT[kt][:, 128 * st:128 * (st + 1)],
                                wt[:, 512 * fh:512 * (fh + 1)],
                                start=(kt == 0), stop=(kt == 7))
                for s2 in range(2):
                    st = 2 * stg + s2
                    for fh in range(2):
                        nc.scalar.copy(
                            v_sb[st][:, 512 * fh:512 * (fh + 1)],
                            psl[2 * s2 + fh][:])

        for i in range(8):
            nc.sync.dma_start(out=pv_sb[i][:],
                              in_=pv_d[128 * i:128 * (i + 1), :])
        for i in range(8):
            nc.sync.dma_start(out=wo_sb[i][:],
                              in_=wo_d[128 * i:128 * (i + 1), :])

        # ---------------- Phase 3: attention ----------------
        def va_sl(j, h, fb):
            src = pv_sb[j] if j < 8 else v_sb[j - 8]
            c0 = DH * h + 128 * fb
            return src[:, c0:c0 + 128]

        with tc.tile_pool(name="p3pj", bufs=4) as prp, \
             tc.tile_pool(name="p3sm", bufs=2) as smp, \
             tc.tile_pool(name="p3pacc", bufs=2) as pap, \
             tc.tile_pool(name="p3sc", bufs=2, space="PSUM") as scps, \
             tc.tile_pool(name="p3y", bufs=2, space="PSUM") as yps, \
             tc.tile_pool(name="p3aux", bufs=1, space="PSUM") as auxp, \
             tc.tile_pool(name="p4o", bufs=2) as osp:
            for TH in range(2):
                for h in range(4):
                    jmax = 12 + 4 * TH
                    ytp = [yps.tile([128, 512], f32, name=f"ytp{fb}",
                                    tag=f"ytp{fb}") for fb in range(2)]
                    pacc = pap.tile([128, 512], f32r, name="pacc", tag="pacc")
                    for j in range(jmax):
                        qs = max(0, 128 * (j - 8) - 512 * TH)
                        W = 512 - qs
                        q0 = 512 * TH + qs
                        sc = scps.tile([128, 512], f32, name="sc", tag="sc")
                        for fk in range(2):
                            nc.tensor.matmul(
                                sc[:, 0:W],
                                kT[2 * h + fk][:, 128 * j:128 * (j + 1)],
                                qh[2 * h + fk][:, q0:q0 + W],
                                start=(fk == 0), stop=(fk == 1))
                        pj = prp.tile([128, 512], bf16, name="pj", tag="pj")
                        nc.scalar.activation(pj[:, 0:W], sc[:, 0:W], AF.Exp,
                                             scale=float(DH ** -0.5))
                        d = 128 * (j - 8) - 512 * TH
                        if 0 <= d < 512:
                            nc.gpsimd.tensor_tensor(pj[:, 0:128], pj[:, 0:128],
                                                    tri[:], op=OP.mult)
                        for fb in range(2):
                            nc.tensor.matmul(
                                ytp[fb][:, qs:512],
                                va_sl(j, h, fb),
                                pj[:, 0:W],
                                start=(j == 0), stop=(j == jmax - 1))
                        with nc.allow_low_precision(reason="denominator acc is f32"):
                            if j == 0:
                                nc.vector.tensor_scalar_mul(
                                    pacc[:], pj[:], 1.0)
                            else:
                                nc.vector.tensor_tensor(
                                    pacc[:, qs:512], pacc[:, qs:512],
                                    pj[:, 0:W], op=OP.add)
                    den = auxp.tile([1, 512], f32, name="den", tag="den")
                    nc.tensor.matmul(den[:], ones_sb[:], pacc[:],
                                     start=True, stop=True)
                    rc = smp.tile([1, 512], f32, name="rc", tag="rc")
                    nc.vector.reciprocal_approx_fast(rc[:], den[:])
                    bc = auxp.tile([128, 512], f32, name="bc", tag="bc")
                    nc.tensor.matmul(bc[:], onesr_sb[:], rc[:],
                                     start=True, stop=True)
                    bc_sb = smp.tile([128, 512], f32, name="bcsb", tag="bcsb")
                    nc.scalar.copy(bc_sb[:], bc[:])
                    for fb in range(2):
                        nc.vector.tensor_tensor(
                            yT[2 * h + fb][:, 512 * TH:512 * (TH + 1)],
                            ytp[fb][:], bc_sb[:], op=OP.mult)

            # ---------------- Phase 4: o-projection ----------------
            for tt in range(8):
                ot = osp.tile([128, DOUT], f32, name="osb", tag="osb")
                for ds in range(3):
                    ops = scps.tile([128, 512], f32, name="ops", tag="sc")
                    for fk in range(8):
                        nc.tensor.matmul(
                            ops[:, 0:384],
                            yT[fk][:, 128 * tt:128 * (tt + 1)],
                            wo_sb[fk][:, 384 * ds:384 * (ds + 1)],
                            start=(fk == 0), stop=(fk == 7))
                    nc.scalar.copy(ot[:, 384 * ds:384 * (ds + 1)],
                                   ops[:, 0:384])
                nc.sync.dma_start(out=out_d[128 * tt:128 * (tt + 1), :],
                                  in_=ot[:])

    nc.finalize()
    return nc


_NC_CACHE = {}


def run(x, past_k, past_v, wq, wk, wv, wo, trace=False):
    from concourse.bass_utils import run_bass_kernel_spmd
    import ml_dtypes

    bf16 = ml_dtypes.bfloat16
    if "nc" not in _NC_CACHE:
        _NC_CACHE["nc"] = build_kernel()
    nc = _NC_CACHE["nc"]
    consts = _host_constants()
    perm = _perm()

    x = np.asarray(x, np.float32)
    wq_p = np.ascontiguousarray(np.asarray(wq, np.float32)[:, perm]).astype(bf16)
    wk_p = np.ascontiguousarray(np.asarray(wk, np.float32)[:, perm]).astype(bf16)
    wv_b = np.ascontiguousarray(np.asarray(wv, np.float32)).astype(bf16)
    wo_b = np.ascontiguousarray(np.asarray(wo, np.float32)).astype(bf16)

    in_maps = []
    for b in range(NCORES):
        xT = np.ascontiguousarray(x[b].T).astype(bf16)          # [din, t]
        pk = np.asarray(past_k[b], np.float32).reshape(P, DIN)[:, perm]
        pkT = np.ascontiguousarray(pk.T).reshape(8, 128, P).astype(bf16)
        pv = np.asarray(past_v[b], np.float32).reshape(P, DIN).astype(bf16)
        m = {
            "xT": xT, "pkT": pkT, "pv": np.ascontiguousarray(pv),
            "wq": wq_p, "wk": wk_p, "wv": wv_b, "wo": wo_b,
            "cos": consts["cos"], "sin": consts["sin"],
            "tri": consts["tri"].astype(bf16),
            "ones": consts["ones"], "onesr": consts["onesr"],
        }
        in_maps.append(m)
    res = run_bass_kernel_spmd(nc, in_maps, list(range(NCORES)), trace=trace)
    out = np.stack([res.results[b]["out"] for b in range(NCORES)], axis=0)
    return out, res


def kernel(x, past_k, past_v, wq, wk, wv, wo):
    out, _ = run(x, past_k, past_v, wq, wk, wv, wo)
    return out


# revision 16
# speedup vs baseline: 1.5625x; 1.0712x over previous
"""Bass/Trainium2 kernel for nn_BakaAttention: 8-way data-parallel over batch.

Per core (one batch element):
  q = rope(x@wq, off=1024); k = rope(concat(past_k, x@wk), off=0); v = x@wv
  out = softmax(mask(q k^T / 16)) [past_v; v] @ wo

Host-side prep (outside HW time): x pre-transposed to [din, t]; wq/wk
columns and past_k features permuted so rope interleaved pairs (2m,2m+1)
land at row m of adjacent feature tiles -> rope is pure elementwise DVE
work with one shared cos/sin table, no PE rotation. All matmul operands
cast to bf16 (streams at 1 col/cycle like f32r, halves SBUF/DMA).

On chip: everything SBUF-resident. Scores computed transposed [keys, q]
so probs feed PV directly as the moving operand; softmax denominators
accumulate on the Vector engine (pacc += pj) with a single [128,1]-ones
matmul per group instead of a per-chunk PE row-sum. Causal structure is
exploited at 128-query granularity: key chunk j only streams the queries
that attend to it, and only the diagonal 128x128 block gets masked.
"""

import numpy as np

B, T, P, H, DH, DIN, DOUT = 8, 1024, 1024, 4, 256, 1024, 1152
S = P + T  # 2048 keys
THETA = 10000.0
NCORES = 8


def _host_constants():
    m = np.arange(128, dtype=np.float64)
    inv = 1.0 / (THETA ** (2.0 * m / DH))                   # [128]
    pos = np.arange(S, dtype=np.float64)                    # [2048]
    ang = np.outer(inv, pos)                                # [128, 2048]
    tri = (np.arange(128)[:, None] <= np.arange(128)[None, :]).astype(np.float32)
    return {
        "cos": np.cos(ang).astype(np.float32),
        "sin": np.sin(ang).astype(np.float32),
        "tri": tri,  # cast to bf16 at pack time
        "ones": np.ones((128, 1), np.float32),
        "onesr": np.ones((1, 128), np.float32),
    }


def _perm():
    # per-head feature permutation: [evens, odds]
    p = np.empty(DIN, np.int64)
    for h in range(H):
        base = DH * h
        p[base:base + 128] = base + 2 * np.arange(128)
        p[base + 128:base + 256] = base + 2 * np.arange(128) + 1
    return p


def build_kernel():
    import concourse.bass as bass
    import concourse.mybir as mybir
    from concourse import bacc
    from concourse.tile import TileContext

    f32 = mybir.dt.float32
    f32r = mybir.dt.float32r
    bf16 = mybir.dt.bfloat16
    AF = mybir.ActivationFunctionType
    OP = mybir.AluOpType

    nc = bacc.Bacc(None, target_bir_lowering=False)

    xT_d = nc.dram_tensor("xT", [DIN, T], bf16, kind="ExternalInput")
    pkT_d = nc.dram_tensor("pkT", [8, 128, P], bf16, kind="ExternalInput")
    pv_d = nc.dram_tensor("pv", [P, DIN], bf16, kind="ExternalInput")
    wq_d = nc.dram_tensor("wq", [DIN, DIN], bf16, kind="ExternalInput")
    wk_d = nc.dram_tensor("wk", [DIN, DIN], bf16, kind="ExternalInput")
    wv_d = nc.dram_tensor("wv", [DIN, DIN], bf16, kind="ExternalInput")
    wo_d = nc.dram_tensor("wo", [DIN, DOUT], bf16, kind="ExternalInput")
    cos_d = nc.dram_tensor("cos", [128, S], f32, kind="ExternalInput")
    sin_d = nc.dram_tensor("sin", [128, S], f32, kind="ExternalInput")
    tri_d = nc.dram_tensor("tri", [128, 128], bf16, kind="ExternalInput")
    ones_d = nc.dram_tensor("ones", [128, 1], f32r, kind="ExternalInput")
    onesr_d = nc.dram_tensor("onesr", [1, 128], f32, kind="ExternalInput")
    out_d = nc.dram_tensor("out", [T, DOUT], f32, kind="ExternalOutput")

    from contextlib import ExitStack
    stack = ExitStack()
    with TileContext(nc) as tc, stack:
        # ---------------- persistent SBUF ----------------
        cstp = stack.enter_context(tc.tile_pool(name="consts", bufs=1))
        cos_t = cstp.tile([128, S], f32, name="cos", tag="cos")
        sin_t = cstp.tile([128, S], f32, name="sin", tag="sin")
        tri = cstp.tile([128, 128], bf16, name="tri", tag="tri")
        ones_sb = cstp.tile([128, 1], f32r, name="ones", tag="ones")
        onesr_sb = cstp.tile([1, 128], f32, name="onesr", tag="onesr")
        nc.sync.dma_start(out=cos_t[:], in_=cos_d[:])
        nc.sync.dma_start(out=sin_t[:], in_=sin_d[:])
        nc.sync.dma_start(out=tri[:], in_=tri_d[:])
        nc.sync.dma_start(out=ones_sb[:], in_=ones_d[:])
        nc.sync.dma_start(out=onesr_sb[:], in_=onesr_d[:])

        resid = stack.enter_context(tc.tile_pool(name="resid", bufs=1))
        xT = [resid.tile([128, T], bf16, name=f"xT{i}", tag=f"xT{i}")
              for i in range(8)]
        kT = [resid.tile([128, S], bf16, name=f"kT{i}", tag=f"kT{i}")
              for i in range(8)]
        qh = [resid.tile([128, T], bf16, name=f"qh{i}", tag=f"qh{i}")
              for i in range(8)]
        v_sb = [resid.tile([128, DIN], bf16, name=f"v{i}", tag=f"v{i}")
                for i in range(8)]
        pv_sb = [resid.tile([128, DIN], bf16, name=f"pv{i}", tag=f"pv{i}")
                 for i in range(8)]
        pkraw = [resid.tile([128, P], bf16, name=f"pkr{i}", tag=f"pkr{i}")
                 for i in range(8)]
        yT = [resid.tile([128, T], bf16, name=f"yT{i}", tag=f"yT{i}")
              for i in range(8)]
        wo_sb = [resid.tile([128, DOUT], bf16, name=f"wo{i}", tag=f"wo{i}")
                 for i in range(8)]

        for i in range(8):
            nc.sync.dma_start(out=xT[i][:], in_=xT_d[128 * i:128 * (i + 1), :])
        for i in range(8):
            nc.sync.dma_start(out=pkraw[i][:], in_=pkT_d[i])

        # past-k rope on gpsimd (independent of PE; runs under projections)
        # pair (A=tile 2h, B=tile 2h+1): kA = A*cos - B*sin; kB = B*cos + A*sin
        def past_rope(h):
            A, Bt = pkraw[2 * h], pkraw[2 * h + 1]
            c = cos_t[:, 0:P]
            s = sin_t[:, 0:P]
            t1 = ropep.tile([128, P], f32, name="prt1", tag="prt1")
            t2 = ropep.tile([128, P], f32, name="prt2", tag="prt2")
            nc.gpsimd.tensor_tensor(t1[:], A[:], c, op=OP.mult)
            nc.gpsimd.tensor_tensor(t2[:], Bt[:], s, op=OP.mult)
            nc.gpsimd.tensor_tensor(kT[2 * h][:, 0:P], t1[:], t2[:],
                                    op=OP.subtract)
            nc.gpsimd.tensor_tensor(t1[:], Bt[:], c, op=OP.mult)
            nc.gpsimd.tensor_tensor(t2[:], A[:], s, op=OP.mult)
            nc.gpsimd.tensor_tensor(kT[2 * h + 1][:, 0:P], t1[:], t2[:],
                                    op=OP.add)

        # ---------------- Phase 1: q/k proj + rope ----------------
        with tc.tile_pool(name="p1w", bufs=3) as wtp, \
             tc.tile_pool(name="p1rope", bufs=1) as ropep, \
             tc.tile_pool(name="p1stage", bufs=2) as stgp, \
             tc.tile_pool(name="p1ps", bufs=2, space="PSUM") as ps1:

            def qk_proj(w_d, dst, doff, do_past):
                for ftg in range(4):  # pair of f-tiles (one head)
                    psl = [ps1.tile([128, 512], f32, name=f"pj{i}",
                                    tag=f"pj{i}") for i in range(4)]
                    for kt in range(8):
                        wt = wtp.tile([128, 256], bf16, name="wld", tag="wld")
                        nc.sync.dma_start(
                            out=wt[:],
                            in_=w_d[128 * kt:128 * (kt + 1),
                                    256 * ftg:256 * (ftg + 1)])
                        for f2 in range(2):
                            for th in range(2):
                                nc.tensor.matmul(
                                    psl[2 * f2 + th][:],
                                    wt[:, 128 * f2:128 * (f2 + 1)],
                                    xT[kt][:, 512 * th:512 * (th + 1)],
                                    start=(kt == 0), stop=(kt == 7))
                    # rope combine: vector in f32, scalar downcasts to bf16
                    c = cos_t[:, P:P + T]
                    s = sin_t[:, P:P + T]
                    for th in range(2):
                        sl = slice(512 * th, 512 * (th + 1))
                        A, Bt = psl[th][:], psl[2 + th][:]
                        t1 = ropep.tile([128, 512], f32, name="rt1", tag="rt1")
                        t2 = ropep.tile([128, 512], f32, name="rt2", tag="rt2")
                        r0 = stgp.tile([128, 512], f32, name="rr0", tag="rr0")
                        nc.vector.tensor_tensor(t1[:], A, c[:, sl], op=OP.mult)
                        nc.vector.tensor_tensor(t2[:], Bt, s[:, sl], op=OP.mult)
                        nc.vector.tensor_tensor(r0[:], t1[:], t2[:],
                                                op=OP.subtract)
                        nc.scalar.copy(
                            dst[2 * ftg][:, doff + 512 * th:doff + 512 * (th + 1)],
                            r0[:])
                        t3 = ropep.tile([128, 512], f32, name="rt3", tag="rt3")
                        t4 = ropep.tile([128, 512], f32, name="rt4", tag="rt4")
                        r1 = stgp.tile([128, 512], f32, name="rr1", tag="rr1")
                        nc.vector.tensor_tensor(t3[:], Bt, c[:, sl], op=OP.mult)
                        nc.vector.tensor_tensor(t4[:], A, s[:, sl], op=OP.mult)
                        nc.vector.tensor_tensor(r1[:], t3[:], t4[:], op=OP.add)
                        nc.scalar.copy(
                            dst[2 * ftg + 1][:, doff + 512 * th:doff + 512 * (th + 1)],
                            r1[:])
                    # interleave past-k rope (gpsimd) among q-proj groups
                    if do_past:
                        past_rope(ftg)

            def v_proj():
                for stg in range(4):
                    psl = [ps1.tile([128, 512], f32, name=f"pv{i}",
                                    tag=f"pj{i}") for i in range(4)]
                    for kt in range(8):
                        wt = wtp.tile([128, 1024], bf16, name="wvld",
                                      tag="wvld")
                        nc.sync.dma_start(out=wt[:],
                                          in_=wv_d[128 * kt:128 * (kt + 1), :])
                        for s2 in range(2):
                            st = 2 * stg + s2
                            for fh in range(2):
                                nc.tensor.matmul(
                                    psl[2 * s2 + fh][:],
                                    xT[kt][:, 128 * st:128 * (st + 1)],
                                    wt[:, 512 * fh:512 * (fh + 1)],
                                    start=(kt == 0), stop=(kt == 7))
                    for s2 in range(2):
                        st = 2 * stg + s2
                        for fh in range(2):
                            nc.scalar.copy(
                                v_sb[st][:, 512 * fh:512 * (fh + 1)],
                                psl[2 * s2 + fh][:])

            qk_proj(wq_d, qh, 0, do_past=True)
            v_proj()  # rope-free: lets vector drain between q and k rope
            qk_proj(wk_d, kT, P, do_past=False)

        for i in range(8):
            nc.sync.dma_start(out=pv_sb[i][:],
                              in_=pv_d[128 * i:128 * (i + 1), :])
        for i in range(8):
            nc.sync.dma_start(out=wo_sb[i][:],
                              in_=wo_d[128 * i:128 * (i + 1), :])

        # ---------------- Phase 3: attention ----------------
        def va_sl(j, h, fb):
            src = pv_sb[j] if j < 8 else v_sb[j - 8]
            c0 = DH * h + 128 * fb
            return src[:, c0:c0 + 128]

        with tc.tile_pool(name="p3pj", bufs=4) as prp, \
             tc.tile_pool(name="p3sm", bufs=2) as smp, \
             tc.tile_pool(name="p3pacc", bufs=2) as pap, \
             tc.tile_pool(name="p3sc", bufs=2, space="PSUM") as scps, \
             tc.tile_pool(name="p3y", bufs=2, space="PSUM") as yps, \
             tc.tile_pool(name="p3aux", bufs=1, space="PSUM") as auxp, \
             tc.tile_pool(name="p4o", bufs=2) as osp:
            for TH in range(2):
                for h in range(4):
                    jmax = 12 + 4 * TH
                    ytp = [yps.tile([128, 512], f32, name=f"ytp{fb}",
                                    tag=f"ytp{fb}") for fb in range(2)]
                    pacc = pap.tile([128, 512], f32r, name="pacc", tag="pacc")
                    for j in range(jmax):
                        qs = max(0, 128 * (j - 8) - 512 * TH)
                        W = 512 - qs
                        q0 = 512 * TH + qs
                        sc = scps.tile([128, 512], f32, name="sc", tag="sc")
                        for fk in range(2):
                            nc.tensor.matmul(
                                sc[:, 0:W],
                                kT[2 * h + fk][:, 128 * j:128 * (j + 1)],
                                qh[2 * h + fk][:, q0:q0 + W],
                                start=(fk == 0), stop=(fk == 1))
                        pj = prp.tile([128, 512], bf16, name="pj", tag="pj")
                        nc.scalar.activation(pj[:, 0:W], sc[:, 0:W], AF.Exp,
                                             scale=float(DH ** -0.5))
                        d = 128 * (j - 8) - 512 * TH
                        if 0 <= d < 512:
                            nc.gpsimd.tensor_tensor(pj[:, 0:128], pj[:, 0:128],
                                                    tri[:], op=OP.mult)
                        for fb in range(2):
                            nc.tensor.matmul(
                                ytp[fb][:, qs:512],
                                va_sl(j, h, fb),
                                pj[:, 0:W],
                                start=(j == 0), stop=(j == jmax - 1))
                        with nc.allow_low_precision(reason="denominator acc is f32"):
                            if j == 0:
                                nc.vector.tensor_scalar_mul(
                                    pacc[:], pj[:], 1.0)
                            else:
                                nc.vector.tensor_tensor(
                                    pacc[:, qs:512], pacc[:, qs:512],
                                    pj[:, 0:W], op=OP.add)
                    den = auxp.tile([1, 512], f32, name="den", tag="den")
                    nc.tensor.matmul(den[:], ones_sb[:], pacc[:],
                                     start=True, stop=True)
                    rc = smp.tile([1, 512], f32, name="rc", tag="rc")
                    nc.vector.reciprocal_approx_fast(rc[:], den[:])
                    bc = auxp.tile([128, 512], f32, name="bc", tag="bc")
                    nc.tensor.matmul(bc[:], onesr_sb[:], rc[:],
                                     start=True, stop=True)
                    bc_sb = smp.tile([128, 512], f32, name="bcsb", tag="bcsb")
                    nc.scalar.copy(bc_sb[:], bc[:])
                    for fb in range(2):
                        nc.vector.tensor_tensor(
                            yT[2 * h + fb][:, 512 * TH:512 * (TH + 1)],
                            ytp[fb][:], bc_sb[:], op=OP.mult)

            # ---------------- Phase 4: o-projection ----------------
            for tt in range(8):
                ot = osp.tile([128, DOUT], f32, name="osb", tag="osb")
                for ds in range(3):
                    ops = scps.tile([128, 512], f32, name="ops", tag="sc")
                    for fk in range(8):
                        nc.tensor.matmul(
                            ops[:, 0:384],
                            yT[fk][:, 128 * tt:128 * (tt + 1)],
                            wo_sb[fk][:, 384 * ds:384 * (ds + 1)],
                            start=(fk == 0), stop=(fk == 7))
                    nc.scalar.copy(ot[:, 384 * ds:384 * (ds + 1)],
                                   ops[:, 0:384])
                nc.sync.dma_start(out=out_d[128 * tt:128 * (tt + 1), :],
                                  in_=ot[:])

    nc.finalize()
    return nc


_NC_CACHE = {}


def run(x, past_k, past_v, wq, wk, wv, wo, trace=False):
    from concourse.bass_utils import run_bass_kernel_spmd
    import ml_dtypes

    bf16 = ml_dtypes.bfloat16
    if "nc" not in _NC_CACHE:
        _NC_CACHE["nc"] = build_kernel()
    nc = _NC_CACHE["nc"]
    consts = _host_constants()
    perm = _perm()

    x = np.asarray(x, np.float32)
    wq_p = np.ascontiguousarray(np.asarray(wq, np.float32)[:, perm]).astype(bf16)
    wk_p = np.ascontiguousarray(np.asarray(wk, np.float32)[:, perm]).astype(bf16)
    wv_b = np.ascontiguousarray(np.asarray(wv, np.float32)).astype(bf16)
    wo_b = np.ascontiguousarray(np.asarray(wo, np.float32)).astype(bf16)

    in_maps = []
    for b in range(NCORES):
        xT = np.ascontiguousarray(x[b].T).astype(bf16)          # [din, t]
        pk = np.asarray(past_k[b], np.float32).reshape(P, DIN)[:, perm]
        pkT = np.ascontiguousarray(pk.T).reshape(8, 128, P).astype(bf16)
        pv = np.asarray(past_v[b], np.float32).reshape(P, DIN).astype(bf16)
        m = {
            "xT": xT, "pkT": pkT, "pv": np.ascontiguousarray(pv),
            "wq": wq_p, "wk": wk_p, "wv": wv_b, "wo": wo_b,
            "cos": consts["cos"], "sin": consts["sin"],
            "tri": consts["tri"].astype(bf16),
            "ones": consts["ones"], "onesr": consts["onesr"],
        }
        in_maps.append(m)
    res = run_bass_kernel_spmd(nc, in_maps, list(range(NCORES)), trace=trace)
    out = np.stack([res.results[b]["out"] for b in range(NCORES)], axis=0)
    return out, res


def kernel(x, past_k, past_v, wq, wk, wv, wo):
    out, _ = run(x, past_k, past_v, wq, wk, wv, wo)
    return out
